# revision 1
# baseline (speedup 1.0000x reference)
"""CARAFE content-aware upsampling kernel for 8 Trainium2 NeuronCores.

Problem: x (4,256,64,64) f32 -> out (4,256,128,128) f32.
  comp = 1x1 conv (256->64), BN(eval)+SiLU, 3x3 conv (64->100),
  softmax over 25 taps, per-pixel 5x5 weighted reassembly at 2x upscale.

Sharding: pure data parallel, 8 shards = 4 batches x 2 row-halves (32 rows),
with the 2-row halo handled by host-side padding. SPMD: one program, per-core
data.

Per-core dataflow, two phases:
  Phase 1 (masks): row-block act tiles (8 output rows + 3x3 halo) so the
  first mask lands before the whole input: PE runs compression conv (K=256,
  BN scale folded into the weights), the 3x3 encoder conv as 9
  PSUM-accumulated matmuls, and a transpose-with-permuted-identity matmul
  that both moves masks to pixel-major and reorders channels k*4+s ->
  s*25+k; ACT applies Sigmoid (BN shift as bias) and Exp (no max-subtract:
  logits are bounded); DVE finishes SiLU, applies the image-validity mask,
  and reduces the softmax denominators Z. Normalization by 1/Z happens on
  the HOST after the run (out and Z ship separately), which removes a
  reciprocal + per-tile scaled-merge serialization from the device.

  Phase 2 (reassembly, the throughput wall ~52M MAC/core): pixels on
  partitions so each mask value is a per-partition scalar. Per 128-pixel
  tile (2 image rows), 25 shifted x slabs stream in pixel-major from a
  host-transposed copy (one DMA each; row-pair slabs are reused across
  adjacent tiles, 10 new per tile). Taps 0..15: DVE fused
  scalar_tensor_tensor (acc = slab*m + acc) - TensorScalarPtr is DVE-only
  on real HW. Taps 16..24: ACT forms mask-scaled products (Copy with
  per-partition scale) and GPSIMD accumulates them with plain tensor_add.
  DVE merges the two accumulators and the result DMAs out pixel-major; the
  host divides by Z and un-transposes.

TimelineSim estimate ~386 us/core; engines balanced (DVE ~330 us busy,
GPSIMD ~310, ACT ~240, PE ~60, HWDGE ~130).
"""

import numpy as np

B, C, H, W = 4, 256, 64, 64
COMP = 64
SCALE, K_UP, K_ENC = 2, 5, 3
EPS = 1e-5
NCORES = 8
HS = H // 2          # 32 rows per core
PR = HS + 4          # 36 padded rows per shard
PCW = W + 4          # 68 padded cols
NPIX = HS * W        # 2048 output-res pixels per core
NACT = (HS + 2) * PCW  # 34*68 = 2312 act pixels (1-row halo for 3x3 conv)
NT = NPIX // 128     # 16 reassembly tiles (2 image rows each)

_cache = {}


def _build(num_devices=NCORES):
    from contextlib import ExitStack

    import concourse.bacc as bacc
    import concourse.bass as bass
    import concourse.mybir as mybir
    import concourse.tile as tile

    import os
    KSPLIT = int(os.environ.get("K_KSPLIT", "16"))
    BUFS = int(os.environ.get("K_BUFS", "2"))
    SLABDMA = os.environ.get("K_SLABDMA", "sp")
    OUTDMA = os.environ.get("K_OUTDMA", "sp")

    f32 = mybir.dt.float32
    nc = bacc.Bacc("TRN2", target_bir_lowering=False, debug=False,
                   num_devices=num_devices)

    xc = nc.dram_tensor("xc", (2, 128, PR * PCW), f32, kind="ExternalInput").ap()
    xt = nc.dram_tensor("xt", (PR * PCW, C), f32, kind="ExternalInput").ap()
    w_eff = nc.dram_tensor("w_eff", (2, 128, COMP), f32, kind="ExternalInput").ap()
    b_eff = nc.dram_tensor("b_eff", (COMP, 1), f32, kind="ExternalInput").ap()
    w_enc9 = nc.dram_tensor("w_enc9", (COMP, 9 * 100), f32, kind="ExternalInput").ap()
    perm = nc.dram_tensor("perm", (100, 100), f32, kind="ExternalInput").ap()
    vmask = nc.dram_tensor("vmask", (NACT,), f32, kind="ExternalInput").ap()
    out_t = nc.dram_tensor("out_t", (NPIX, 4, C), f32, kind="ExternalOutput").ap()
    z_out = nc.dram_tensor("z_out", (128, NT, 4), f32, kind="ExternalOutput").ap()

    mult = mybir.AluOpType.mult
    add = mybir.AluOpType.add
    AF = mybir.ActivationFunctionType

    with tile.TileContext(nc) as tc, ExitStack() as ctx:
        const = ctx.enter_context(tc.tile_pool(name="const", bufs=1))
        work = ctx.enter_context(tc.tile_pool(name="work", bufs=2))
        psA = ctx.enter_context(tc.tile_pool(name="psA", bufs=2, space="PSUM"))
        psB = ctx.enter_context(tc.tile_pool(name="psB", bufs=2, space="PSUM"))
        psC = ctx.enter_context(tc.tile_pool(name="psC", bufs=min(2 * BUFS, 4),
                                             space="PSUM"))

        # ---- resident constants ----
        w_eff_s = []
        for h in range(2):
            t = const.tile([128, COMP], f32, tag=f"weff{h}")
            nc.sync.dma_start(out=t, in_=w_eff[h])
            w_eff_s.append(t)
        b_eff_s = const.tile([COMP, 1], f32, tag="beff")
        nc.sync.dma_start(out=b_eff_s, in_=b_eff)
        w_enc_s = const.tile([COMP, 9 * 100], f32, tag="wenc")
        nc.sync.dma_start(out=w_enc_s, in_=w_enc9)
        perm_s = const.tile([100, 100], f32, tag="perm")
        nc.sync.dma_start(out=perm_s, in_=perm)
        vm_s = const.tile([COMP, NACT], f32, tag="vm")
        nc.scalar.dma_start(
            out=vm_s,
            in_=bass.AP(tensor=vmask.tensor, offset=vmask.offset,
                        ap=[[0, COMP]] + list(vmask.ap)),
        )

        # ---- per 8-row chunk: row-block input load, compression conv,
        #      encoder conv, exp, transpose, denominators; then per 2-row
        #      tile: reassembly. Row-block act tiles (10 rows = 8 output
        #      rows + 3x3 halo) keep the first mask chunk off the critical
        #      path of the whole input load. ----
        ACHUNK = 10 * PCW
        xt3 = xt.rearrange("(r c) d -> r c d", c=PCW)

        # ---- Phase 1: all masks. Row-block act tiles (10 rows = 8 output
        # rows + 3x3 halo); PE does compression + encoder convs + transposes
        # in one warm burst; ACT exponentiates and copies masks to SBUF;
        # DVE reduces softmax denominators (shipped to host, which divides).
        mks = []
        zsall = const.tile([128, NT, 4], f32, tag="zsall")
        # small first/last chunks: first masks land sooner (PE is cold at
        # start), last chunk drains earlier into the reassembly tail
        chunks = [(0, 4), (4, 8), (12, 8), (20, 8), (28, 4)]
        for i0, nr in chunks:
            arows = nr + 2
            apix = arows * PCW
            xb = []
            for h in range(2):
                t_xb = work.tile([128, ACHUNK], f32, tag=f"xb{h}", bufs=2)
                eng = nc.sync if h == 0 else nc.scalar
                eng.dma_start(
                    out=t_xb[:, :apix],
                    in_=xc[h][:, (i0 + 1) * PCW:(i0 + 1 + arows) * PCW])
                xb.append(t_xb)
            ac = work.tile([COMP, ACHUNK], f32, tag="ac", bufs=2)
            nsub = (apix + 339) // 340
            for ci in range(nsub):
                n0 = ci * 340
                n = min(340, apix - n0)
                pc = psA.tile([COMP, 340], f32, tag="pc")
                for h in range(2):
                    nc.tensor.matmul(
                        pc[:, :n], w_eff_s[h], xb[h][:, n0:n0 + n],
                        start=(h == 0), stop=(h == 1),
                    )
                sg = work.tile([COMP, 340], f32, tag="sg")
                nc.scalar.activation(out=sg[:, :n], in_=pc[:, :n],
                                     func=AF.Sigmoid, bias=b_eff_s, scale=1.0)
                # act = (comp+shift)*sigmoid(comp+shift), then validity mask
                nc.vector.scalar_tensor_tensor(
                    out=ac[:, n0:n0 + n], in0=pc[:, :n], scalar=b_eff_s,
                    in1=sg[:, :n], op0=add, op1=mult)
                nc.vector.tensor_mul(
                    ac[:, n0:n0 + n], ac[:, n0:n0 + n],
                    vm_s[:, i0 * PCW + n0:i0 * PCW + n0 + n])
            ac3 = ac[:, :apix].rearrange("p (r c) -> p r c", c=PCW)

            pm = psB.tile([100, 512], f32, tag="pm")
            npx = nr * 64
            for idx in range(9):
                ky, kx = divmod(idx, 3)
                rhs = ac3[:, ky:ky + nr, kx + 1:kx + 65]
                nc.tensor.matmul(
                    pm[:, :npx], w_enc_s[:, idx * 100:(idx + 1) * 100], rhs,
                    start=(idx == 0), stop=(idx == 8),
                )
            exp_s = work.tile([100, 512], f32, tag="exp")
            nc.scalar.activation(out=exp_s[:, :npx], in_=pm[:, :npx],
                                 func=AF.Exp)

            for q in range(nr // 2):
                t = i0 // 2 + q
                pt = psC.tile([128, 100], f32, tag="pt")
                nc.tensor.matmul(pt, exp_s[:, q * 128:(q + 1) * 128], perm_s,
                                 start=True, stop=True)
                mk = work.tile([128, 100], f32, tag="mk", bufs=17)
                nc.scalar.activation(out=mk, in_=pt, func=AF.Copy)
                nc.vector.reduce_sum(
                    out=zsall[:, t, :],
                    in_=pt[:].rearrange("p (s k) -> p s k", k=25),
                    axis=mybir.AxisListType.X,
                )
                mks.append(mk)

        nc.sync.dma_start(out=z_out, in_=zsall)

        # ---- Phase 2: reassembly. Taps 0..KSPLIT-1 on DVE (fused
        # TensorScalarPtr, DVE-only op on real HW). Taps KSPLIT..24: ACT
        # forms the mask-scaled product (Copy with per-partition scale) and
        # GPSIMD accumulates with plain tensor_add.
        slab_cache = {}
        for t in range(NT):
            mk = mks[t]
            slabs = []
            for k25 in range(25):
                dy, dx = divmod(k25, 5)
                key = (2 * t + dy, dx)
                R = slab_cache.get(key)
                if R is None:
                    R = work.tile([128, C], f32, tag="slab", bufs=48)
                    if SLABDMA == "gpsimd":
                        eng = nc.gpsimd
                    elif SLABDMA == "sp":
                        eng = nc.sync
                    else:
                        eng = nc.sync if (dy + dx) % 2 == 0 else nc.scalar
                    eng.dma_start(out=R[:],
                                  in_=xt3[key[0]:key[0] + 2, dx:dx + 64, :])
                    slab_cache[key] = R
                slabs.append(R)

            accD = work.tile([128, 4, C], f32, tag="accD", bufs=BUFS)
            accG = work.tile([128, 4, C], f32, tag="accG", bufs=BUFS)
            for s in range(4):
                nc.vector.tensor_scalar_mul(
                    out=accD[:, s], in0=slabs[0], scalar1=mk[:, s * 25:s * 25 + 1]
                )
                for k25 in range(1, KSPLIT):
                    nc.vector.scalar_tensor_tensor(
                        out=accD[:, s], in0=slabs[k25],
                        scalar=mk[:, s * 25 + k25:s * 25 + k25 + 1],
                        in1=accD[:, s], op0=mult, op1=add,
                    )
                for k25 in range(KSPLIT, 25):
                    col = mk[:, s * 25 + k25:s * 25 + k25 + 1]
                    if k25 == KSPLIT:
                        nc.scalar.activation(out=accG[:, s], in_=slabs[k25],
                                             func=AF.Copy, scale=col)
                    else:
                        prod = work.tile([128, C], f32, tag="prod", bufs=8)
                        nc.scalar.activation(out=prod, in_=slabs[k25],
                                             func=AF.Copy, scale=col)
                        nc.gpsimd.tensor_add(accG[:, s], accG[:, s], prod)
            # softmax normalization deferred to host (divide by Z)
            if KSPLIT < 25:
                nc.vector.tensor_add(accD[:], accD[:], accG[:])
            oeng = nc.gpsimd if OUTDMA == "gpsimd" else nc.sync
            oeng.dma_start(out=out_t[t * 128:(t + 1) * 128], in_=accD)

    nc.compile()
    return nc


def _host_inputs(x, w_comp, bn_gamma, bn_beta, bn_mean, bn_var, w_enc):
    inv = (bn_gamma / np.sqrt(bn_var + EPS)).astype(np.float32)
    w_eff = (w_comp * inv[:, None]).T.astype(np.float32)          # (256,64)
    w_eff = np.ascontiguousarray(w_eff.reshape(2, 128, COMP))
    b_eff = (bn_beta - bn_mean * inv).astype(np.float32).reshape(COMP, 1)
    w_enc9 = np.ascontiguousarray(
        w_enc.transpose(1, 2, 3, 0).reshape(COMP, 9 * 100).astype(np.float32))
    perm = np.zeros((100, 100), np.float32)
    for k in range(25):
        for s in range(4):
            perm[k * 4 + s, s * 25 + k] = 1.0

    xp = np.pad(x.astype(np.float32), ((0, 0), (0, 0), (2, 2), (2, 2)))
    in_maps = []
    for core in range(NCORES):
        b, half = divmod(core, 2)
        h0 = HS * half
        sh = xp[b, :, h0:h0 + PR, :]                              # (256,36,68)
        xc = np.ascontiguousarray(sh.reshape(2, 128, PR * PCW))
        xt = np.ascontiguousarray(sh.transpose(1, 2, 0).reshape(PR * PCW, C))
        ar = h0 - 1 + np.arange(HS + 2)
        vr = (ar >= 0) & (ar < H)
        acj = np.arange(PCW) - 2
        vc = (acj >= 0) & (acj < W)
        vmask = (vr[:, None] & vc[None, :]).astype(np.float32).reshape(NACT)
        in_maps.append({"xc": xc, "xt": xt, "w_eff": w_eff, "b_eff": b_eff,
                        "w_enc9": w_enc9, "perm": perm, "vmask": vmask})
    return in_maps


def _run(nc, in_maps, **kw):
    from concourse import bass_utils
    return bass_utils.run_bass_kernel_spmd(nc, in_maps,
                                           core_ids=list(range(NCORES)), **kw)


def kernel(x, w_comp, bn_gamma, bn_beta, bn_mean, bn_var, w_enc):
    if "nc" not in _cache:
        _cache["nc"] = _build()
    in_maps = _host_inputs(np.asarray(x, np.float32), np.asarray(w_comp),
                           np.asarray(bn_gamma), np.asarray(bn_beta),
                           np.asarray(bn_mean), np.asarray(bn_var),
                           np.asarray(w_enc))
    res = _run(_cache["nc"], in_maps)
    out = np.zeros((B, C, H * SCALE, W * SCALE), np.float32)
    for core in range(NCORES):
        b, half = divmod(core, 2)
        h0 = HS * half
        ot = res.results[core]["out_t"]                           # (2048,4,256)
        z = res.results[core]["z_out"].transpose(1, 0, 2).reshape(NPIX, 4)
        ot = ot / z[:, :, None]
        o = (ot.reshape(HS, W, 2, 2, C).transpose(4, 0, 2, 1, 3)
               .reshape(C, HS * 2, W * 2))
        out[b, :, h0 * 2:h0 * 2 + HS * 2, :] = o
    return out



# revision 2
# speedup vs baseline: 731.5951x; 731.5951x over previous
"""CARAFE content-aware upsampling kernel for 8 Trainium2 NeuronCores.

Problem: x (4,256,64,64) f32 -> out (4,256,128,128) f32.
  comp = 1x1 conv (256->64), BN(eval)+SiLU, 3x3 conv (64->100),
  softmax over 25 taps, per-pixel 5x5 weighted reassembly at 2x upscale.

Sharding: pure data parallel, 8 shards = 4 batches x 2 row-halves (32 rows),
with the 2-row halo handled by host-side padding. SPMD: one program, per-core
data.

Per-core dataflow, two phases:
  Phase 1 (masks): row-block act tiles (8 output rows + 3x3 halo) so the
  first mask lands before the whole input: PE runs compression conv (K=256,
  BN scale folded into the weights), the 3x3 encoder conv as 9
  PSUM-accumulated matmuls, and a transpose-with-permuted-identity matmul
  that both moves masks to pixel-major and reorders channels k*4+s ->
  s*25+k; ACT applies Sigmoid (BN shift as bias) and Exp (no max-subtract:
  logits are bounded); DVE finishes SiLU, applies the image-validity mask,
  and reduces the softmax denominators Z. Normalization by 1/Z happens on
  the HOST after the run (out and Z ship separately), which removes a
  reciprocal + per-tile scaled-merge serialization from the device.

  Phase 2 (reassembly, the throughput wall ~52M MAC/core): pixels on
  partitions so each mask value is a per-partition scalar. Per 128-pixel
  tile (2 image rows), 25 shifted x slabs stream in pixel-major from a
  host-transposed copy (one DMA each; row-pair slabs are reused across
  adjacent tiles, 10 new per tile). Taps 0..15: DVE fused
  scalar_tensor_tensor (acc = slab*m + acc) - TensorScalarPtr is DVE-only
  on real HW. Taps 16..24: ACT forms mask-scaled products (Copy with
  per-partition scale) and GPSIMD accumulates them with plain tensor_add.
  DVE merges the two accumulators and the result DMAs out pixel-major; the
  host divides by Z and un-transposes.

TimelineSim estimate ~386 us/core; engines balanced (DVE ~330 us busy,
GPSIMD ~310, ACT ~240, PE ~60, HWDGE ~130).
"""

import numpy as np

B, C, H, W = 4, 256, 64, 64
COMP = 64
SCALE, K_UP, K_ENC = 2, 5, 3
EPS = 1e-5
NCORES = 8
HS = H // 2          # 32 rows per core
PR = HS + 4          # 36 padded rows per shard
PCW = W + 4          # 68 padded cols
NPIX = HS * W        # 2048 output-res pixels per core
NACT = (HS + 2) * PCW  # 34*68 = 2312 act pixels (1-row halo for 3x3 conv)
NT = NPIX // 128     # 16 reassembly tiles (2 image rows each)

_cache = {}


def _build(num_devices=NCORES):
    from contextlib import ExitStack

    import concourse.bacc as bacc
    import concourse.bass as bass
    import concourse.mybir as mybir
    import concourse.tile as tile

    import os
    KSPLIT = int(os.environ.get("K_KSPLIT", "16"))
    BUFS = int(os.environ.get("K_BUFS", "2"))
    SLABDMA = os.environ.get("K_SLABDMA", "sp")
    OUTDMA = os.environ.get("K_OUTDMA", "sp")

    f32 = mybir.dt.float32
    nc = bacc.Bacc("TRN2", target_bir_lowering=False, debug=False,
                   num_devices=num_devices)

    xc = nc.dram_tensor("xc", (2, 128, PR * PCW), f32, kind="ExternalInput").ap()
    xt = nc.dram_tensor("xt", (PR * PCW, C), f32, kind="ExternalInput").ap()
    w_eff = nc.dram_tensor("w_eff", (2, 128, COMP), f32, kind="ExternalInput").ap()
    b_eff = nc.dram_tensor("b_eff", (COMP, 1), f32, kind="ExternalInput").ap()
    w_enc9 = nc.dram_tensor("w_enc9", (COMP, 9 * 100), f32, kind="ExternalInput").ap()
    perm = nc.dram_tensor("perm", (100, 100), f32, kind="ExternalInput").ap()
    vmask = nc.dram_tensor("vmask", (NACT,), f32, kind="ExternalInput").ap()
    out_t = nc.dram_tensor("out_t", (NPIX, 4, C), f32, kind="ExternalOutput").ap()
    z_out = nc.dram_tensor("z_out", (128, NT, 4), f32, kind="ExternalOutput").ap()

    mult = mybir.AluOpType.mult
    add = mybir.AluOpType.add
    AF = mybir.ActivationFunctionType

    with tile.TileContext(nc) as tc, ExitStack() as ctx:
        const = ctx.enter_context(tc.tile_pool(name="const", bufs=1))
        work = ctx.enter_context(tc.tile_pool(name="work", bufs=2))
        psA = ctx.enter_context(tc.tile_pool(name="psA", bufs=2, space="PSUM"))
        psB = ctx.enter_context(tc.tile_pool(name="psB", bufs=2, space="PSUM"))
        psC = ctx.enter_context(tc.tile_pool(name="psC", bufs=min(2 * BUFS, 4),
                                             space="PSUM"))

        # ---- resident constants ----
        w_eff_s = []
        for h in range(2):
            t = const.tile([128, COMP], f32, tag=f"weff{h}")
            nc.sync.dma_start(out=t, in_=w_eff[h])
            w_eff_s.append(t)
        b_eff_s = const.tile([COMP, 1], f32, tag="beff")
        nc.sync.dma_start(out=b_eff_s, in_=b_eff)
        w_enc_s = const.tile([COMP, 9 * 100], f32, tag="wenc")
        nc.sync.dma_start(out=w_enc_s, in_=w_enc9)
        perm_s = const.tile([100, 100], f32, tag="perm")
        nc.sync.dma_start(out=perm_s, in_=perm)
        vm_s = const.tile([COMP, NACT], f32, tag="vm")
        nc.scalar.dma_start(
            out=vm_s,
            in_=bass.AP(tensor=vmask.tensor, offset=vmask.offset,
                        ap=[[0, COMP]] + list(vmask.ap)),
        )

        # ---- per 8-row chunk: row-block input load, compression conv,
        #      encoder conv, exp, transpose, denominators; then per 2-row
        #      tile: reassembly. Row-block act tiles (10 rows = 8 output
        #      rows + 3x3 halo) keep the first mask chunk off the critical
        #      path of the whole input load. ----
        ACHUNK = 10 * PCW
        xt3 = xt.rearrange("(r c) d -> r c d", c=PCW)

        # ---- Phase 1: all masks. Row-block act tiles (10 rows = 8 output
        # rows + 3x3 halo); PE does compression + encoder convs + transposes
        # in one warm burst; ACT exponentiates and copies masks to SBUF;
        # DVE reduces softmax denominators (shipped to host, which divides).
        mks = []
        zsall = const.tile([128, NT, 4], f32, tag="zsall")
        # small first/last chunks: first masks land sooner (PE is cold at
        # start), last chunk drains earlier into the reassembly tail
        chunks = [(0, 4), (4, 8), (12, 8), (20, 8), (28, 4)]
        for i0, nr in chunks:
            arows = nr + 2
            apix = arows * PCW
            xb = []
            for h in range(2):
                t_xb = work.tile([128, ACHUNK], f32, tag=f"xb{h}", bufs=2)
                eng = nc.sync if h == 0 else nc.scalar
                eng.dma_start(
                    out=t_xb[:, :apix],
                    in_=xc[h][:, (i0 + 1) * PCW:(i0 + 1 + arows) * PCW])
                xb.append(t_xb)
            ac = work.tile([COMP, ACHUNK], f32, tag="ac", bufs=2)
            nsub = (apix + 339) // 340
            for ci in range(nsub):
                n0 = ci * 340
                n = min(340, apix - n0)
                pc = psA.tile([COMP, 340], f32, tag="pc")
                for h in range(2):
                    nc.tensor.matmul(
                        pc[:, :n], w_eff_s[h], xb[h][:, n0:n0 + n],
                        start=(h == 0), stop=(h == 1),
                    )
                sg = work.tile([COMP, 340], f32, tag="sg")
                nc.scalar.activation(out=sg[:, :n], in_=pc[:, :n],
                                     func=AF.Sigmoid, bias=b_eff_s, scale=1.0)
                # act = (comp+shift)*sigmoid(comp+shift), then validity mask
                nc.vector.scalar_tensor_tensor(
                    out=ac[:, n0:n0 + n], in0=pc[:, :n], scalar=b_eff_s,
                    in1=sg[:, :n], op0=add, op1=mult)
                nc.vector.tensor_mul(
                    ac[:, n0:n0 + n], ac[:, n0:n0 + n],
                    vm_s[:, i0 * PCW + n0:i0 * PCW + n0 + n])
            ac3 = ac[:, :apix].rearrange("p (r c) -> p r c", c=PCW)

            pm = psB.tile([100, 512], f32, tag="pm")
            npx = nr * 64
            for idx in range(9):
                ky, kx = divmod(idx, 3)
                rhs = ac3[:, ky:ky + nr, kx + 1:kx + 65]
                nc.tensor.matmul(
                    pm[:, :npx], w_enc_s[:, idx * 100:(idx + 1) * 100], rhs,
                    start=(idx == 0), stop=(idx == 8),
                )
            exp_s = work.tile([100, 512], f32, tag="exp")
            nc.scalar.activation(out=exp_s[:, :npx], in_=pm[:, :npx],
                                 func=AF.Exp)

            for q in range(nr // 2):
                t = i0 // 2 + q
                pt = psC.tile([128, 100], f32, tag="pt")
                nc.tensor.matmul(pt, exp_s[:, q * 128:(q + 1) * 128], perm_s,
                                 start=True, stop=True)
                mk = work.tile([128, 100], f32, tag="mk", bufs=17)
                nc.scalar.activation(out=mk, in_=pt, func=AF.Copy)
                nc.vector.reduce_sum(
                    out=zsall[:, t, :],
                    in_=pt[:].rearrange("p (s k) -> p s k", k=25),
                    axis=mybir.AxisListType.X,
                )
                mks.append(mk)

        nc.sync.dma_start(out=z_out, in_=zsall)

        # ---- Phase 2: reassembly. Taps 0..KSPLIT-1 on DVE (fused
        # TensorScalarPtr, DVE-only op on real HW). Taps KSPLIT..24: ACT
        # forms the mask-scaled product (Copy with per-partition scale) and
        # GPSIMD accumulates with plain tensor_add.
        slab_cache = {}
        for t in range(NT):
            mk = mks[t]
            slabs = []
            for k25 in range(25):
                dy, dx = divmod(k25, 5)
                key = (2 * t + dy, dx)
                R = slab_cache.get(key)
                if R is None:
                    R = work.tile([128, C], f32, tag="slab", bufs=48)
                    if SLABDMA == "gpsimd":
                        eng = nc.gpsimd
                    elif SLABDMA == "sp":
                        eng = nc.sync
                    else:
                        eng = nc.sync if (dy + dx) % 2 == 0 else nc.scalar
                    eng.dma_start(out=R[:],
                                  in_=xt3[key[0]:key[0] + 2, dx:dx + 64, :])
                    slab_cache[key] = R
                slabs.append(R)

            accD = work.tile([128, 4, C], f32, tag="accD", bufs=BUFS)
            accG = work.tile([128, 4, C], f32, tag="accG", bufs=BUFS)
            for s in range(4):
                nc.vector.tensor_scalar_mul(
                    out=accD[:, s], in0=slabs[0], scalar1=mk[:, s * 25:s * 25 + 1]
                )
                for k25 in range(1, KSPLIT):
                    nc.vector.scalar_tensor_tensor(
                        out=accD[:, s], in0=slabs[k25],
                        scalar=mk[:, s * 25 + k25:s * 25 + k25 + 1],
                        in1=accD[:, s], op0=mult, op1=add,
                    )
                for k25 in range(KSPLIT, 25):
                    col = mk[:, s * 25 + k25:s * 25 + k25 + 1]
                    if k25 == KSPLIT:
                        nc.scalar.activation(out=accG[:, s], in_=slabs[k25],
                                             func=AF.Copy, scale=col)
                    else:
                        prod = work.tile([128, C], f32, tag="prod", bufs=8)
                        nc.scalar.activation(out=prod, in_=slabs[k25],
                                             func=AF.Copy, scale=col)
                        nc.gpsimd.tensor_add(accG[:, s], accG[:, s], prod)
            # softmax normalization deferred to host (divide by Z)
            if KSPLIT < 25:
                nc.vector.tensor_add(accD[:], accD[:], accG[:])
            oeng = nc.gpsimd if OUTDMA == "gpsimd" else nc.sync
            oeng.dma_start(out=out_t[t * 128:(t + 1) * 128], in_=accD)

    nc.compile()
    return nc


def _host_inputs(x, w_comp, bn_gamma, bn_beta, bn_mean, bn_var, w_enc):
    inv = (bn_gamma / np.sqrt(bn_var + EPS)).astype(np.float32)
    w_eff = (w_comp * inv[:, None]).T.astype(np.float32)          # (256,64)
    w_eff = np.ascontiguousarray(w_eff.reshape(2, 128, COMP))
    b_eff = (bn_beta - bn_mean * inv).astype(np.float32).reshape(COMP, 1)
    w_enc9 = np.ascontiguousarray(
        w_enc.transpose(1, 2, 3, 0).reshape(COMP, 9 * 100).astype(np.float32))
    perm = np.zeros((100, 100), np.float32)
    for k in range(25):
        for s in range(4):
            perm[k * 4 + s, s * 25 + k] = 1.0

    xp = np.pad(x.astype(np.float32), ((0, 0), (0, 0), (2, 2), (2, 2)))
    in_maps = []
    for core in range(NCORES):
        b, half = divmod(core, 2)
        h0 = HS * half
        sh = xp[b, :, h0:h0 + PR, :]                              # (256,36,68)
        xc = np.ascontiguousarray(sh.reshape(2, 128, PR * PCW))
        xt = np.ascontiguousarray(sh.transpose(1, 2, 0).reshape(PR * PCW, C))
        ar = h0 - 1 + np.arange(HS + 2)
        vr = (ar >= 0) & (ar < H)
        acj = np.arange(PCW) - 2
        vc = (acj >= 0) & (acj < W)
        vmask = (vr[:, None] & vc[None, :]).astype(np.float32).reshape(NACT)
        in_maps.append({"xc": xc, "xt": xt, "w_eff": w_eff, "b_eff": b_eff,
                        "w_enc9": w_enc9, "perm": perm, "vmask": vmask})
    return in_maps


class _Results:
    def __init__(self, results):
        self.results = results


def _get_runner(nc):
    """Persistent compiled executable for nc (compile once, reuse forever).

    Replaces bass_utils.run_bass_kernel_spmd, which rebuilds the jax.jit
    closure on every call (re-trace + re-lower + executable load over the
    axon tunnel) and ships ~64MB of zero-init output buffers host->device
    per call. This kernel writes every element of every output, so the
    output-operand buffers are never read: create them on-device once and
    skip donation entirely.
    """
    if "runner" in _cache:
        return _cache["runner"]
    import jax
    import jax.numpy as jnp
    from jax.experimental.shard_map import shard_map
    from jax.sharding import Mesh, NamedSharding, PartitionSpec

    from concourse import bass2jax
    import concourse.mybir as mybir

    bass2jax.install_neuronx_cc_hook()

    partition_name = (nc.partition_id_tensor.name
                      if nc.partition_id_tensor else None)
    in_names, out_names, out_avals = [], [], []
    for alloc in nc.m.functions[0].allocations:
        if not isinstance(alloc, mybir.MemoryLocationSet):
            continue
        name = alloc.memorylocations[0].name
        if alloc.kind == "ExternalInput":
            if name != partition_name:
                in_names.append(name)
        elif alloc.kind == "ExternalOutput":
            out_names.append(name)
            out_avals.append(jax.core.ShapedArray(
                tuple(alloc.tensor_shape), mybir.dt.np(alloc.dtype)))
    n_params = len(in_names)
    n_outs = len(out_names)
    all_in = list(in_names) + list(out_names)
    if partition_name is not None:
        all_in.append(partition_name)

    def _body(*args):
        operands = list(args)
        if partition_name is not None:
            operands.append(bass2jax.partition_id_tensor())
        outs = bass2jax._bass_exec_p.bind(
            *operands,
            out_avals=tuple(out_avals),
            in_names=tuple(all_in),
            out_names=tuple(out_names),
            lowering_input_output_aliases=(),
            sim_require_finite=True,
            sim_require_nnan=True,
            nc=nc,
        )
        return tuple(outs)

    devices = jax.devices()[:NCORES]
    mesh = Mesh(np.asarray(devices), ("core",))
    spec = PartitionSpec("core")
    sharded = jax.jit(
        shard_map(_body, mesh=mesh, in_specs=(spec,) * (n_params + n_outs),
                  out_specs=(spec,) * n_outs, check_rep=False),
        keep_unused=True)
    sh = NamedSharding(mesh, spec)
    dummies = jax.jit(
        lambda: tuple(jnp.zeros((NCORES * a.shape[0], *a.shape[1:]), a.dtype)
                      for a in out_avals),
        out_shardings=(sh,) * n_outs)()
    jax.block_until_ready(dummies)
    runner = dict(sharded=sharded, in_names=in_names, out_names=out_names,
                  out_avals=out_avals, sh=sh, dummies=dummies)
    _cache["runner"] = runner
    return runner


def _stage(runner, in_maps):
    """Concat per-core inputs and place them on the 8 cores (axis 0 sharded)."""
    import jax
    staged = []
    for name in runner["in_names"]:
        a = np.concatenate([np.asarray(m[name]) for m in in_maps], axis=0)
        staged.append(jax.device_put(a, runner["sh"]))
    jax.block_until_ready(staged)
    return staged


def _launch(runner, staged):
    """One async execute on all 8 cores; returns device arrays (no fetch)."""
    return runner["sharded"](*staged, *runner["dummies"])


def _fetch(runner, outs):
    host = [np.asarray(o) for o in outs]
    results = []
    for c in range(NCORES):
        results.append({
            name: host[i].reshape(NCORES, *runner["out_avals"][i].shape)[c]
            for i, name in enumerate(runner["out_names"])})
    return _Results(results)


def _run(nc, in_maps, **kw):
    runner = _get_runner(nc)
    return _fetch(runner, _launch(runner, _stage(runner, in_maps)))


def kernel(x, w_comp, bn_gamma, bn_beta, bn_mean, bn_var, w_enc):
    if "nc" not in _cache:
        _cache["nc"] = _build()
    in_maps = _host_inputs(np.asarray(x, np.float32), np.asarray(w_comp),
                           np.asarray(bn_gamma), np.asarray(bn_beta),
                           np.asarray(bn_mean), np.asarray(bn_var),
                           np.asarray(w_enc))
    res = _run(_cache["nc"], in_maps)
    out = np.zeros((B, C, H * SCALE, W * SCALE), np.float32)
    for core in range(NCORES):
        b, half = divmod(core, 2)
        h0 = HS * half
        ot = res.results[core]["out_t"]                           # (2048,4,256)
        z = res.results[core]["z_out"].transpose(1, 0, 2).reshape(NPIX, 4)
        ot = ot / z[:, :, None]
        o = (ot.reshape(HS, W, 2, 2, C).transpose(4, 0, 2, 1, 3)
               .reshape(C, HS * 2, W * 2))
        out[b, :, h0 * 2:h0 * 2 + HS * 2, :] = o
    return out

